# revision 1
# baseline (speedup 1.0000x reference)
"""Bass/Trainium2 kernel for nn_BipartiteGNN: 6 rounds of bipartite LSTM
message passing + output projection, distributed over 8 NeuronCores.

Distribution:
 - Edges/vertices sharded 8 ways (contiguous blocks). The gathered h
   tables are replicated per core in a subrange-permuted row layout
   (row v -> j*25000 + rank*3125 + i%3125, j=i//3125) so each of the
   chunked AllGathers rebuilds one contiguous 25000-row gather bucket
   (int16 gather indices) and overlaps with compute.
 - segment_sum: dma_gather of source rows (entries on partitions, 256B
   fp16 rows) + one-hot selection matmul per 128-entry chunk,
   accumulating into a column-major [128(D), 512] PSUM msg window. The
   selection matrix is built on DVE via tensor_scalar is_equal against
   an iota row (4x fp16 mode).
 - LSTM runs column-major (D on partitions) in fp16 with f32 PSUM; c
   state is fp16, SBUF-resident per phase. h is written back both
   column-major (local recurrent input) and, via PE transpose,
   row-major into the AllGather input shard.
"""
import sys
import numpy as np

sys.path.insert(0, '/opt/trn_rl_repo')

import concourse.bacc as bacc  # noqa: E402
import concourse.mybir as mybir  # noqa: E402
from concourse.tile import TileContext  # noqa: E402
from concourse.masks import make_identity  # noqa: E402
from concourse import bass_utils  # noqa: E402

P = 128
D = 128
V = 100000
E = 200000
VOCAB = 1000
ITERS = 6
CORES = 8
W = 512

EK = E // CORES        # 25000
VK = V // CORES        # 12500
BUCKET = 25000         # rows per gather bucket in both tables

NB_E = 4               # buckets over the (permuted) vertex table
NB_V = 8               # buckets over the (permuted) edge table
SUB_V = VK // NB_E     # 3125
SUB_E = EK // NB_V     # 3125
WG_E = 6
WG_V = 4

NW_E = (EK + W - 1) // W   # 49
NW_V = (VK + W - 1) // W   # 25

f16 = mybir.dt.float16
f32 = mybir.dt.float32
i16 = mybir.dt.int16

AF = mybir.ActivationFunctionType
ALU = mybir.AluOpType

_CACHE = {}
DEBUG_STAGE = 'full'


# --------------------------------------------------------------------------
# host-side index preprocessing
# --------------------------------------------------------------------------

def _perm_pos(glob, shard, nb):
    sub = shard // nb
    rank = glob // shard
    i = glob - rank * shard
    j = i // sub
    return j * (CORES * sub) + rank * sub + (i - j * sub)


def _wrap16(vals, ncols):
    buf = np.zeros(ncols * 16, np.int16)
    buf[:len(vals)] = vals
    return np.tile(buf.reshape(ncols, 16).T, (8, 1))


def _prep_phase(tgt, src, n_t_local, src_shard, nb, nw, wg):
    src_perm = _perm_pos(src, src_shard, nb)
    bucket = src_perm // BUCKET
    sloc = (src_perm - bucket * BUCKET).astype(np.int64)

    percore = []
    for k in range(CORES):
        m = (tgt // n_t_local) == k
        t = tgt[m] - k * n_t_local
        b = bucket[m]
        s = sloc[m]
        w = t // W
        order = np.lexsort((t, w, b))
        percore.append((b[order], w[order], t[order], s[order]))

    sizes = np.zeros((CORES, nb, nw), np.int64)
    for k in range(CORES):
        b, w, _, _ = percore[k]
        np.add.at(sizes[k], (b, w), 1)
    ncc = (sizes.max(axis=0) + P - 1) // P
    for w in range(nw):
        if ncc[:, w].sum() == 0:
            ncc[0, w] = 1

    ng = (nw + wg - 1) // wg
    gblocks = np.zeros((nb, ng), np.int64)
    for b in range(nb):
        for g in range(ng):
            gblocks[b, g] = ncc[b, g * wg:(g + 1) * wg].sum()
    maxblk = int(gblocks.max())

    idx_off = np.zeros((nb, ng), np.int64)
    o = 0
    for b in range(nb):
        for g in range(ng):
            idx_off[b, g] = o
            o += 8 * int(gblocks[b, g])
    idx_cols = int(o)

    tl_off = np.zeros((nb, nw), np.int64)
    o = 0
    for w in range(nw):
        for b in range(nb):
            tl_off[b, w] = o
            o += int(ncc[b, w])
    tl_cols = int(o)

    blk_off = np.zeros((nb, nw), np.int64)
    for b in range(nb):
        for g in range(ng):
            o = 0
            for w in range(g * wg, min((g + 1) * wg, nw)):
                blk_off[b, w] = o
                o += int(ncc[b, w])

    idx_arrs, tl_arrs = [], []
    for k in range(CORES):
        b_a, w_a, t_a, s_a = percore[k]
        idx_full = np.zeros(idx_cols * 16, np.int16)
        tl_full = np.full(tl_cols * P, -1.0, np.float32)
        cell_start = {}
        pos = 0
        for b in range(nb):
            for w in range(nw):
                cell_start[(b, w)] = pos
                pos += int(sizes[k, b, w])
        for b in range(nb):
            for g in range(ng):
                base = int(idx_off[b, g]) * 16
                cur = 0
                for w in range(g * wg, min((g + 1) * wg, nw)):
                    cs = cell_start[(b, w)]
                    cnt = int(sizes[k, b, w])
                    L = int(ncc[b, w]) * P
                    if L == 0:
                        continue
                    sl = s_a[cs:cs + cnt]
                    padded = np.full(L, sl[0] if cnt else 0, np.int16)
                    padded[:cnt] = sl
                    idx_full[base + cur: base + cur + L] = padded
                    tlv = np.full(L, -1.0, np.float32)
                    tlv[:cnt] = (t_a[cs:cs + cnt] - w * W).astype(np.float32)
                    tb = int(tl_off[b, w]) * P
                    tl_full[tb:tb + L] = tlv
                    cur += L
        idx_arrs.append(np.tile(idx_full.reshape(idx_cols, 16).T, (8, 1)))
        tl_arrs.append(np.ascontiguousarray(tl_full.reshape(tl_cols, P).T))

    return dict(
        ncc=ncc, ng=ng, wg=wg, nb=nb, nw=nw, maxblk=maxblk,
        gblocks=gblocks, idx_off=idx_off, idx_cols=idx_cols,
        tl_off=tl_off, tl_cols=tl_cols, blk_off=blk_off,
        idx_arrs=idx_arrs, tl_arrs=tl_arrs, n_t_local=n_t_local,
    )


# init chunks for building hv_tab (perm layout): (start_row, nrows)
INIT_CHUNKS = [(i * 12800, 12800) for i in range(7)] + [(89600, 10400)]


def _prep_init(x_v):
    """idx arrays (into emb) for hv_tab init gathers, in perm order."""
    perm = _perm_pos(np.arange(V, dtype=np.int64), VK, NB_E)
    inv = np.empty(V, np.int64)
    inv[perm] = np.arange(V, dtype=np.int64)
    xv_of_pos = x_v[inv]                      # emb row for table position p
    pieces = []
    for start, nrows in INIT_CHUNKS:
        nblk = (nrows + P - 1) // P
        vals = np.full(nblk * P, 0, np.int16)
        vals[:nrows] = xv_of_pos[start:start + nrows]
        if nrows < nblk * P:
            vals[nrows:] = vals[nrows - 1]
        pieces.append(vals)
    allv = np.concatenate(pieces)
    ncols = len(allv) // 16
    return _wrap16(allv, ncols), ncols


# --------------------------------------------------------------------------
# device program
# --------------------------------------------------------------------------

def _store_rows(nc, dram, row0, nrows, sb3):
    """[128, nblk, 128] fp16 tile -> dram rows [row0, row0+nrows)."""
    nbf = nrows // P
    rem = nrows - nbf * P
    if nbf:
        nc.sync.dma_start(
            out=dram[row0:row0 + nbf * P, :].rearrange("(b p) d -> p b d", p=P),
            in_=sb3[:, :nbf, :])
    if rem:
        nc.sync.dma_start(
            out=dram[row0 + nbf * P: row0 + nrows, :],
            in_=sb3[:rem, nbf, :])


def _build_phase(nc, pools, cfg, meta, it, first, last_v):
    sb, spool, msgps, zps, trps, cpool_c = (
        pools['sb'], pools['s'], pools['msg'], pools['z'], pools['tr'],
        pools['c'])
    gpool = pools['g' + cfg['tag']]
    NT = meta['n_t_local']
    nw, nb, wg = meta['nw'], meta['nb'], meta['wg']
    ncc, blk_off, tl_off = meta['ncc'], meta['blk_off'], meta['tl_off']
    gblocks, idx_off = meta['gblocks'], meta['idx_off']
    tab, h_col, c_col = cfg['tab'], cfg['h_col'], cfg['c_col']
    row_shard = cfg['row_shard']
    idx_sb, tl_sb = cfg['idx_sb'], cfg['tl_sb']
    wih_sb, whh_sb, bias_sb = cfg['wih_sb'], cfg['whh_sb'], cfg['bias_sb']
    iota_sb, ident_sb = cfg['iota_sb'], cfg['ident_sb']
    tg = cfg['tag']

    c_sb = cpool_c.tile([P, EK], f16, tag="c_res", name=f"c{tg}{it}")
    if not first:
        nc.sync.dma_start(out=c_sb[:, :NT], in_=c_col[:, :])

    g_tiles = {}
    for w in range(nw):
        T = min(W, NT - w * W)
        g = w // wg
        if w % wg == 0:
            for b in range(nb):
                nblk = int(gblocks[b, g])
                if nblk == 0:
                    continue
                gt = gpool.tile([P, meta['maxblk'], P], f16,
                                tag=f"g{tg}{b}", name=f"g{tg}{b}_{it}_{g}")
                io = int(idx_off[b, g])
                nc.gpsimd.dma_gather(
                    out_ap=gt[:, :nblk, :],
                    in_ap=tab[b * BUCKET:(b + 1) * BUCKET, :],
                    idxs_ap=idx_sb[:, io:io + 8 * nblk],
                    num_idxs=nblk * P, num_idxs_reg=nblk * P,
                    elem_size=D, single_packet=(nblk * P <= 1024))
                g_tiles[b] = gt

        msg_ps = msgps.tile([P, W], f32, tag="msg", name=f"m{tg}{it}_{w}")
        nch = int(ncc[:, w].sum())
        ci = 0
        for b in range(nb):
            for j in range(int(ncc[b, w])):
                s_t = spool.tile([P, W], f16, tag="s",
                                 name=f"s{tg}{it}_{w}_{ci}")
                nc.vector.tensor_scalar(
                    out=s_t[:, :T], in0=iota_sb[:, :T],
                    scalar1=tl_sb[:, int(tl_off[b, w]) + j, None],
                    scalar2=None, op0=ALU.is_equal)
                nc.tensor.matmul(
                    out=msg_ps[:, :T],
                    lhsT=g_tiles[b][:, int(blk_off[b, w]) + j, :],
                    rhs=s_t[:, :T],
                    start=(ci == 0), stop=(ci == nch - 1))
                ci += 1

        msg_sb = sb.tile([P, W], f16, tag="msg_sb", name=f"ms{tg}{it}_{w}")
        nc.scalar.copy(out=msg_sb[:, :T], in_=msg_ps[:, :T])

        hp = sb.tile([P, W], f16, tag="hp", name=f"hp{tg}{it}_{w}")
        nc.sync.dma_start(out=hp[:, :T], in_=h_col[:, w * W:w * W + T])

        gates = []
        for gi, fn in enumerate((AF.Sigmoid, AF.Sigmoid, AF.Tanh, AF.Sigmoid)):
            z_ps = zps.tile([P, W], f32, tag="z", name=f"z{tg}{it}_{w}_{gi}")
            nc.tensor.matmul(out=z_ps[:, :T],
                             lhsT=wih_sb[:, gi * P:(gi + 1) * P],
                             rhs=msg_sb[:, :T], start=True, stop=False)
            nc.tensor.matmul(out=z_ps[:, :T],
                             lhsT=whh_sb[:, gi * P:(gi + 1) * P],
                             rhs=hp[:, :T], start=False, stop=True)
            ga = sb.tile([P, W], f16, tag=f"gate{gi}",
                         name=f"ga{tg}{it}_{w}_{gi}")
            nc.scalar.activation(out=ga[:, :T], in_=z_ps[:, :T], func=fn,
                                 bias=bias_sb[:, gi, None])
            gates.append(ga)
        i_g, f_g, g_g, o_g = gates

        csl = c_sb[:, w * W:w * W + T]
        if first:
            nc.vector.tensor_tensor(out=csl, in0=i_g[:, :T], in1=g_g[:, :T],
                                    op=ALU.mult)
        else:
            fc = sb.tile([P, W], f16, tag="fc", name=f"fc{tg}{it}_{w}")
            nc.vector.tensor_tensor(out=fc[:, :T], in0=f_g[:, :T], in1=csl,
                                    op=ALU.mult)
            ig = sb.tile([P, W], f16, tag="ig", name=f"ig{tg}{it}_{w}")
            nc.vector.tensor_tensor(out=ig[:, :T], in0=i_g[:, :T],
                                    in1=g_g[:, :T], op=ALU.mult)
            nc.vector.tensor_tensor(out=csl, in0=fc[:, :T], in1=ig[:, :T],
                                    op=ALU.add)
        tc_t = sb.tile([P, W], f16, tag="tanc", name=f"tc{tg}{it}_{w}")
        nc.scalar.activation(out=tc_t[:, :T], in_=csl, func=AF.Tanh)
        hn = sb.tile([P, W], f16, tag="hn", name=f"hn{tg}{it}_{w}")
        nc.vector.tensor_tensor(out=hn[:, :T], in0=o_g[:, :T],
                                in1=tc_t[:, :T], op=ALU.mult)
        nc.sync.dma_start(out=h_col[:, w * W:w * W + T], in_=hn[:, :T])

        if not last_v:
            tr_ps = trps.tile([P, W], f16, tag="tr", name=f"tr{tg}{it}_{w}")
            nbk = (T + P - 1) // P
            for j in range(nbk):
                nj = min(P, T - j * P)
                nc.tensor.transpose(out=tr_ps[:nj, j * P:j * P + P],
                                    in_=hn[:, j * P:j * P + nj],
                                    identity=ident_sb[:])
            hrow = sb.tile([P, W // P, P], f16, tag="hrow",
                           name=f"hr{tg}{it}_{w}")
            nc.vector.tensor_copy(
                out=hrow[:, :nbk, :],
                in_=tr_ps[:].rearrange("p (b q) -> p b q", q=P)[:, :nbk, :])
            _store_rows(nc, row_shard, w * W, T, hrow[:])

    if not last_v:
        nc.sync.dma_start(out=c_col[:, :], in_=c_sb[:, :NT])


def build_program(me, mv, xvt_cols):
    nc = bacc.Bacc(num_devices=CORES)

    emb16 = nc.dram_tensor("emb16", [VOCAB + 1, D], f16, kind="ExternalInput")
    evec_in = nc.dram_tensor("evec_in", [P, W], f16, kind="ExternalInput")
    wihe_in = nc.dram_tensor("wihe", [P, 4 * P], f16, kind="ExternalInput")
    whhe_in = nc.dram_tensor("whhe", [P, 4 * P], f16, kind="ExternalInput")
    wihv_in = nc.dram_tensor("wihv", [P, 4 * P], f16, kind="ExternalInput")
    whhv_in = nc.dram_tensor("whhv", [P, 4 * P], f16, kind="ExternalInput")
    biase_in = nc.dram_tensor("biase", [P, 4], f32, kind="ExternalInput")
    biasv_in = nc.dram_tensor("biasv", [P, 4], f32, kind="ExternalInput")
    wout_in = nc.dram_tensor("woutt", [P, VOCAB], f16, kind="ExternalInput")
    bout_in = nc.dram_tensor("boutr", [P, VOCAB], f32, kind="ExternalInput")
    iota_in = nc.dram_tensor("iota", [P, W], f16, kind="ExternalInput")
    eidx_in = nc.dram_tensor("eidx", [P, me['idx_cols']], i16,
                             kind="ExternalInput")
    etl_in = nc.dram_tensor("etl", [P, me['tl_cols']], f32,
                            kind="ExternalInput")
    vidx_in = nc.dram_tensor("vidx", [P, mv['idx_cols']], i16,
                             kind="ExternalInput")
    vtl_in = nc.dram_tensor("vtl", [P, mv['tl_cols']], f32,
                            kind="ExternalInput")
    xvt_in = nc.dram_tensor("xvt", [P, xvt_cols], i16, kind="ExternalInput")
    xvc_cols = 12544 // 16  # 784: 12500 idxs padded to 98 blocks
    xvc_in = nc.dram_tensor("xvc", [P, xvc_cols], i16, kind="ExternalInput")

    logits = nc.dram_tensor("logits", [VK, VOCAB], f32, kind="ExternalOutput")

    hv_tab = nc.dram_tensor("hv_tab", [V, D], f16, addr_space="Shared")
    he_tab = nc.dram_tensor("he_tab", [E, D], f16, addr_space="Shared")
    hv_row = nc.dram_tensor("hv_row", [VK, D], f16)
    he_row = nc.dram_tensor("he_row", [EK, D], f16)
    hv_col = nc.dram_tensor("hv_col", [P, VK], f16)
    he_col = nc.dram_tensor("he_col", [P, EK], f16)
    cv_col = nc.dram_tensor("cv_col", [P, VK], f16)
    ce_col = nc.dram_tensor("ce_col", [P, EK], f16)

    rg = [list(range(CORES))]

    with TileContext(nc) as tc:
        with tc.tile_pool(name="const", bufs=1) as cpool:
            def load_const(name, src, shape, dt):
                t = cpool.tile(shape, dt, name=name)
                nc.sync.dma_start(out=t[:], in_=src[:, :])
                return t

            wihe_sb = load_const("wihe_sb", wihe_in, [P, 4 * P], f16)
            whhe_sb = load_const("whhe_sb", whhe_in, [P, 4 * P], f16)
            wihv_sb = load_const("wihv_sb", wihv_in, [P, 4 * P], f16)
            whhv_sb = load_const("whhv_sb", whhv_in, [P, 4 * P], f16)
            biase_sb = load_const("biase_sb", biase_in, [P, 4], f32)
            biasv_sb = load_const("biasv_sb", biasv_in, [P, 4], f32)
            iota_sb = load_const("iota_sb", iota_in, [P, W], f16)
            eidx_sb = load_const("eidx_sb", eidx_in, [P, me['idx_cols']], i16)
            etl_sb = load_const("etl_sb", etl_in, [P, me['tl_cols']], f32)
            vidx_sb = load_const("vidx_sb", vidx_in, [P, mv['idx_cols']], i16)
            vtl_sb = load_const("vtl_sb", vtl_in, [P, mv['tl_cols']], f32)
            ident_sb = cpool.tile([P, P], f16, name="ident_sb")
            make_identity(nc, ident_sb[:])

            # ---------------- init ----------------
            with tc.tile_pool(name="initp", bufs=2) as ip:
                xvt_sb = ip.tile([P, xvt_cols], i16, name="xvt_sb", bufs=1)
                nc.sync.dma_start(out=xvt_sb[:], in_=xvt_in[:, :])
                col = 0
                for start, nrows in INIT_CHUNKS:
                    nblk = (nrows + P - 1) // P
                    gt = ip.tile([P, 100, P], f16, tag="initg",
                                 name=f"ig{start}")
                    nc.gpsimd.dma_gather(
                        out_ap=gt[:, :nblk, :], in_ap=emb16[:, :],
                        idxs_ap=xvt_sb[:, col:col + 8 * nblk],
                        num_idxs=nblk * P, num_idxs_reg=nblk * P,
                        elem_size=D, single_packet=(nblk * P <= 1024))
                    _store_rows(nc, hv_tab, start, nrows, gt[:])
                    col += 8 * nblk

                xvc_sb = ip.tile([P, xvc_cols], i16, name="xvc_sb", bufs=1)
                nc.sync.dma_start(out=xvc_sb[:], in_=xvc_in[:, :])
                nci = 16 * xvc_cols  # 12544
                hvc0 = ip.tile([P, 1, nci], f16, name="hvc0", bufs=1)
                nc.gpsimd.dma_gather(
                    out_ap=hvc0[:], in_ap=emb16[:, :], idxs_ap=xvc_sb[:],
                    num_idxs=nci, num_idxs_reg=nci, elem_size=D,
                    transpose=True, single_packet=(nci <= 1024))
                nc.sync.dma_start(out=hv_col[:, :], in_=hvc0[:, 0, :VK])

                # he_col: broadcast edge_vec by doubling
                nc.sync.dma_start(out=he_col[:, 0:W], in_=evec_in[:, :])
                span = W
                while span < EK:
                    n = min(span, EK - span)
                    nc.sync.dma_start(out=he_col[:, span:span + n],
                                      in_=he_col[:, 0:n])
                    span += n

            # ---------------- main loop ----------------
            with (
                tc.tile_pool(name="sb", bufs=2) as sb,
                tc.tile_pool(name="spool", bufs=4) as spool,
                tc.tile_pool(name="ge", bufs=2) as ge,
                tc.tile_pool(name="gv", bufs=2) as gv,
                tc.tile_pool(name="cres", bufs=1) as cres,
                tc.tile_pool(name="msgps", bufs=2, space="PSUM") as msgps,
                tc.tile_pool(name="zps", bufs=4, space="PSUM") as zps,
                tc.tile_pool(name="trps", bufs=2, space="PSUM") as trps,
            ):
                pools = dict(sb=sb, s=spool, msg=msgps, z=zps, tr=trps,
                             c=cres, ge=ge, gv=gv)
                cfg_e = dict(tab=hv_tab, h_col=he_col, c_col=ce_col,
                             row_shard=he_row, idx_sb=eidx_sb, tl_sb=etl_sb,
                             wih_sb=wihe_sb, whh_sb=whhe_sb,
                             bias_sb=biase_sb, iota_sb=iota_sb,
                             ident_sb=ident_sb, tag='e')
                cfg_v = dict(tab=he_tab, h_col=hv_col, c_col=cv_col,
                             row_shard=hv_row, idx_sb=vidx_sb, tl_sb=vtl_sb,
                             wih_sb=wihv_sb, whh_sb=whhv_sb,
                             bias_sb=biasv_sb, iota_sb=iota_sb,
                             ident_sb=ident_sb, tag='v')
                stage = DEBUG_STAGE
                for it in range(ITERS):
                    if stage == 'init':
                        break
                    if it > 0:
                        for j in range(NB_E):
                            nc.gpsimd.collective_compute(
                                "AllGather", ALU.bypass, replica_groups=rg,
                                ins=[hv_row[j * SUB_V:(j + 1) * SUB_V, :].opt()],
                                outs=[hv_tab[j * BUCKET:(j + 1) * BUCKET, :].opt()])
                    _build_phase(nc, pools, cfg_e, me, it,
                                 first=(it == 0), last_v=False)
                    if stage == 'edge':
                        break
                    for j in range(NB_V):
                        nc.gpsimd.collective_compute(
                            "AllGather", ALU.bypass, replica_groups=rg,
                            ins=[he_row[j * SUB_E:(j + 1) * SUB_E, :].opt()],
                            outs=[he_tab[j * BUCKET:(j + 1) * BUCKET, :].opt()])
                    if stage == 'agv':
                        break
                    _build_phase(nc, pools, cfg_v, mv, it,
                                 first=(it == 0), last_v=(it == ITERS - 1))

            # ---------------- logits ----------------
            with (
                tc.tile_pool(name="lsb", bufs=3) as lsb,
                tc.tile_pool(name="lcp", bufs=1) as lcp,
                tc.tile_pool(name="lps", bufs=2, space="PSUM") as lps,
            ):
                wout_sb = lcp.tile([P, VOCAB], f16, name="wout_sb")
                nc.sync.dma_start(out=wout_sb[:], in_=wout_in[:, :])
                bout_sb = lcp.tile([P, VOCAB], f32, name="bout_sb")
                nc.sync.dma_start(out=bout_sb[:], in_=bout_in[:, :])
                hvc_sb = lcp.tile([P, VK], f16, name="hvc_sb")
                nc.sync.dma_start(out=hvc_sb[:], in_=hv_col[:, :])
                nch = (VK + P - 1) // P
                for ch in range(nch):
                    n = min(P, VK - ch * P)
                    lp = lps.tile([P, VOCAB], f32, tag="lp", name=f"lp{ch}")
                    nc.tensor.matmul(out=lp[:n, :W],
                                     lhsT=hvc_sb[:, ch * P:ch * P + n],
                                     rhs=wout_sb[:, :W],
                                     start=True, stop=True)
                    nc.tensor.matmul(out=lp[:n, W:VOCAB],
                                     lhsT=hvc_sb[:, ch * P:ch * P + n],
                                     rhs=wout_sb[:, W:VOCAB],
                                     start=True, stop=True)
                    ob = lsb.tile([P, VOCAB], f32, tag="ob", name=f"ob{ch}")
                    nc.vector.tensor_tensor(out=ob[:n, :], in0=lp[:n, :],
                                            in1=bout_sb[:n, :], op=ALU.add)
                    nc.sync.dma_start(out=logits[ch * P:ch * P + n, :],
                                      in_=ob[:n, :])

    nc.compile()
    return nc


# --------------------------------------------------------------------------
# entry point
# --------------------------------------------------------------------------

def _prepare(inputs):
    rows = np.asarray(inputs['adj_rows']).astype(np.int64)
    cols = np.asarray(inputs['adj_cols']).astype(np.int64)
    x_v = np.asarray(inputs['x_v']).astype(np.int64)

    me = _prep_phase(rows, cols, EK, VK, NB_E, NW_E, WG_E)
    mv = _prep_phase(cols, rows, VK, EK, NB_V, NW_V, WG_V)
    xvt_arr, xvt_cols = _prep_init(x_v)

    emb = np.asarray(inputs['emb'], np.float32)
    emb16 = np.ascontiguousarray(emb.astype(np.float16))
    evec = (np.asarray(inputs['edge_init_w'], np.float32)[:, 0]
            + np.asarray(inputs['edge_init_b'], np.float32))
    evec_tile = np.ascontiguousarray(
        np.tile(evec.astype(np.float16)[:, None], (1, W)))

    def wt(name):
        return np.ascontiguousarray(
            np.asarray(inputs[name], np.float32).T.astype(np.float16))

    def bias(ih, hh):
        b = (np.asarray(inputs[ih], np.float32)
             + np.asarray(inputs[hh], np.float32))
        return np.ascontiguousarray(b.reshape(4, P).T)

    wout_t = np.ascontiguousarray(
        np.asarray(inputs['Wout'], np.float32).T.astype(np.float16))
    bout_rep = np.ascontiguousarray(
        np.tile(np.asarray(inputs['bout'], np.float32)[None, :], (P, 1)))
    iota = np.ascontiguousarray(
        np.tile(np.arange(W, dtype=np.float32).astype(np.float16), (P, 1)))

    common = dict(
        emb16=emb16, evec_in=evec_tile,
        wihe=wt('Wih_e'), whhe=wt('Whh_e'),
        wihv=wt('Wih_v'), whhv=wt('Whh_v'),
        biase=bias('bih_e', 'bhh_e'), biasv=bias('bih_v', 'bhh_v'),
        woutt=wout_t, boutr=bout_rep, iota=iota, xvt=xvt_arr,
    )
    in_maps = []
    for k in range(CORES):
        xs = x_v[k * VK:(k + 1) * VK].astype(np.int16)
        xvc = np.concatenate([xs, np.full(12544 - VK, xs[-1], np.int16)])
        m = dict(common)
        m.update(
            eidx=me['idx_arrs'][k], etl=me['tl_arrs'][k],
            vidx=mv['idx_arrs'][k], vtl=mv['tl_arrs'][k],
            xvc=_wrap16(xvc, 12544 // 16),
        )
        in_maps.append(m)
    return me, mv, xvt_cols, in_maps


def run_spmd(inputs, **kw):
    me, mv, xvt_cols, in_maps = _prepare(inputs)
    key = (me['ncc'].tobytes(), mv['ncc'].tobytes(), xvt_cols,
           me['gblocks'].tobytes(), mv['gblocks'].tobytes(),
           ITERS, DEBUG_STAGE)
    if key not in _CACHE:
        _CACHE[key] = build_program(me, mv, xvt_cols)
    nc = _CACHE[key]
    return bass_utils.run_bass_kernel_spmd(
        nc, in_maps, core_ids=list(range(CORES)), **kw)


def kernel(**inputs) -> np.ndarray:
    res = run_spmd(inputs)
    out = np.concatenate([res.results[k]['logits'] for k in range(CORES)], 0)
    return out.astype(np.float32)



# revision 5
# speedup vs baseline: 1.2460x; 1.2460x over previous
"""Bass/Trainium2 kernel for nn_BipartiteGNN: 6 rounds of bipartite LSTM
message passing + output projection, distributed over 8 NeuronCores.

Distribution:
 - Edges/vertices sharded 8 ways (contiguous blocks). The gathered h
   tables are replicated per core in a subrange-permuted row layout
   (row v -> j*25000 + rank*3125 + i%3125, j=i//3125) so each of the
   chunked AllGathers rebuilds one contiguous 25000-row gather bucket
   (int16 gather indices) and overlaps with compute.
 - segment_sum: dma_gather of source rows (entries on partitions, 256B
   fp16 rows) + one-hot selection matmul per 128-entry chunk,
   accumulating into a column-major [128(D), 512] PSUM msg window. The
   selection matrix is built on DVE via tensor_scalar is_equal against
   an iota row (4x fp16 mode).
 - LSTM runs column-major (D on partitions) in fp16 with f32 PSUM; c
   state is fp16, SBUF-resident per phase. h is written back both
   column-major (local recurrent input) and, via PE transpose,
   row-major into the AllGather input shard.
"""
import sys
import numpy as np

sys.path.insert(0, '/opt/trn_rl_repo')

import concourse.bacc as bacc  # noqa: E402
import concourse.mybir as mybir  # noqa: E402
from concourse.tile import TileContext  # noqa: E402
from concourse.masks import make_identity  # noqa: E402
from concourse import bass_utils  # noqa: E402

P = 128
D = 128
V = 100000
E = 200000
VOCAB = 1000
ITERS = 6
CORES = 8
W = 512

EK = E // CORES        # 25000
VK = V // CORES        # 12500
BUCKET = 25000         # rows per gather bucket in both tables

NB_E = 4               # buckets over the (permuted) vertex table
NB_V = 8               # buckets over the (permuted) edge table
SUB_V = VK // NB_E     # 3125
SUB_E = EK // NB_V     # 3125
WG_E = 6
WG_V = 4

NW_E = (EK + W - 1) // W   # 49
NW_V = (VK + W - 1) // W   # 25

f16 = mybir.dt.float16
f32 = mybir.dt.float32
i16 = mybir.dt.int16

AF = mybir.ActivationFunctionType
ALU = mybir.AluOpType

_CACHE = {}
DEBUG_STAGE = 'full'


# --------------------------------------------------------------------------
# host-side index preprocessing
# --------------------------------------------------------------------------

def _perm_pos(glob, shard, nb):
    sub = shard // nb
    rank = glob // shard
    i = glob - rank * shard
    j = i // sub
    return j * (CORES * sub) + rank * sub + (i - j * sub)


def _wrap16(vals, ncols):
    buf = np.zeros(ncols * 16, np.int16)
    buf[:len(vals)] = vals
    return np.tile(buf.reshape(ncols, 16).T, (8, 1))


def _prep_phase(tgt, src, n_t_local, src_shard, nb, nw, wg):
    src_perm = _perm_pos(src, src_shard, nb)
    bucket = src_perm // BUCKET
    sloc = (src_perm - bucket * BUCKET).astype(np.int64)

    percore = []
    for k in range(CORES):
        m = (tgt // n_t_local) == k
        t = tgt[m] - k * n_t_local
        b = bucket[m]
        s = sloc[m]
        w = t // W
        order = np.lexsort((t, w, b))
        percore.append((b[order], w[order], t[order], s[order]))

    sizes = np.zeros((CORES, nb, nw), np.int64)
    for k in range(CORES):
        b, w, _, _ = percore[k]
        np.add.at(sizes[k], (b, w), 1)
    ncc = (sizes.max(axis=0) + P - 1) // P
    for w in range(nw):
        if ncc[:, w].sum() == 0:
            ncc[0, w] = 1

    ng = (nw + wg - 1) // wg
    gblocks = np.zeros((nb, ng), np.int64)
    for b in range(nb):
        for g in range(ng):
            gblocks[b, g] = ncc[b, g * wg:(g + 1) * wg].sum()
    maxblk = int(gblocks.max())

    idx_off = np.zeros((nb, ng), np.int64)
    o = 0
    for b in range(nb):
        for g in range(ng):
            idx_off[b, g] = o
            o += 8 * int(gblocks[b, g])
    idx_cols = int(o)

    tl_off = np.zeros((nb, nw), np.int64)
    o = 0
    for w in range(nw):
        for b in range(nb):
            tl_off[b, w] = o
            o += int(ncc[b, w])
    tl_cols = int(o)

    blk_off = np.zeros((nb, nw), np.int64)
    for b in range(nb):
        for g in range(ng):
            o = 0
            for w in range(g * wg, min((g + 1) * wg, nw)):
                blk_off[b, w] = o
                o += int(ncc[b, w])

    idx_arrs, tl_arrs = [], []
    for k in range(CORES):
        b_a, w_a, t_a, s_a = percore[k]
        idx_full = np.zeros(idx_cols * 16, np.int16)
        tl_full = np.full(tl_cols * P, -1.0, np.float32)
        cell_start = {}
        pos = 0
        for b in range(nb):
            for w in range(nw):
                cell_start[(b, w)] = pos
                pos += int(sizes[k, b, w])
        for b in range(nb):
            for g in range(ng):
                base = int(idx_off[b, g]) * 16
                cur = 0
                for w in range(g * wg, min((g + 1) * wg, nw)):
                    cs = cell_start[(b, w)]
                    cnt = int(sizes[k, b, w])
                    L = int(ncc[b, w]) * P
                    if L == 0:
                        continue
                    sl = s_a[cs:cs + cnt]
                    padded = np.full(L, sl[0] if cnt else 0, np.int16)
                    padded[:cnt] = sl
                    idx_full[base + cur: base + cur + L] = padded
                    tlv = np.full(L, -1.0, np.float32)
                    tlv[:cnt] = (t_a[cs:cs + cnt] - w * W).astype(np.float32)
                    tb = int(tl_off[b, w]) * P
                    tl_full[tb:tb + L] = tlv
                    cur += L
        idx_arrs.append(np.tile(idx_full.reshape(idx_cols, 16).T, (8, 1)))
        tl_arrs.append(np.ascontiguousarray(tl_full.reshape(tl_cols, P).T))

    return dict(
        ncc=ncc, ng=ng, wg=wg, nb=nb, nw=nw, maxblk=maxblk,
        gblocks=gblocks, idx_off=idx_off, idx_cols=idx_cols,
        tl_off=tl_off, tl_cols=tl_cols, blk_off=blk_off,
        idx_arrs=idx_arrs, tl_arrs=tl_arrs, n_t_local=n_t_local,
    )


# init chunks for building hv_tab (perm layout): (start_row, nrows)
INIT_CHUNKS = [(i * 12800, 12800) for i in range(7)] + [(89600, 10400)]


def _prep_init(x_v):
    """idx arrays (into emb) for hv_tab init gathers, in perm order."""
    perm = _perm_pos(np.arange(V, dtype=np.int64), VK, NB_E)
    inv = np.empty(V, np.int64)
    inv[perm] = np.arange(V, dtype=np.int64)
    xv_of_pos = x_v[inv]                      # emb row for table position p
    pieces = []
    for start, nrows in INIT_CHUNKS:
        nblk = (nrows + P - 1) // P
        vals = np.full(nblk * P, 0, np.int16)
        vals[:nrows] = xv_of_pos[start:start + nrows]
        if nrows < nblk * P:
            vals[nrows:] = vals[nrows - 1]
        pieces.append(vals)
    allv = np.concatenate(pieces)
    ncols = len(allv) // 16
    return _wrap16(allv, ncols), ncols


# --------------------------------------------------------------------------
# device program
# --------------------------------------------------------------------------

def _store_rows(nc, dram, row0, nrows, sb3):
    """[128, nblk, 128] fp16 tile -> dram rows [row0, row0+nrows)."""
    nbf = nrows // P
    rem = nrows - nbf * P
    if nbf:
        nc.sync.dma_start(
            out=dram[row0:row0 + nbf * P, :].rearrange("(b p) d -> p b d", p=P),
            in_=sb3[:, :nbf, :])
    if rem:
        nc.sync.dma_start(
            out=dram[row0 + nbf * P: row0 + nrows, :],
            in_=sb3[:rem, nbf, :])


def _build_phase(nc, pools, cfg, meta, it, first, last_v):
    sb, spool, msgps, zps, trps, cpool_c = (
        pools['sb'], pools['s'], pools['msg'], pools['z'], pools['tr'],
        pools['c'])
    gpool = pools['g' + cfg['tag']]
    NT = meta['n_t_local']
    nw, nb, wg = meta['nw'], meta['nb'], meta['wg']
    ncc, blk_off, tl_off = meta['ncc'], meta['blk_off'], meta['tl_off']
    gblocks, idx_off = meta['gblocks'], meta['idx_off']
    tab, h_col, c_col = cfg['tab'], cfg['h_col'], cfg['c_col']
    row_shard = cfg['row_shard']
    idx_sb, tl_sb = cfg['idx_sb'], cfg['tl_sb']
    wih_sb, whh_sb, bias_sb = cfg['wih_sb'], cfg['whh_sb'], cfg['bias_sb']
    iota_sb, ident_sb = cfg['iota_sb'], cfg['ident_sb']
    tg = cfg['tag']

    c_sb = cpool_c.tile([P, EK], f16, tag="c_res", name=f"c{tg}{it}")
    if not first:
        nc.sync.dma_start(out=c_sb[:, :NT], in_=c_col[:, :])

    g_tiles = {}
    for w in range(nw):
        T = min(W, NT - w * W)
        g = w // wg
        if w % wg == 0:
            for b in range(nb):
                nblk = int(gblocks[b, g])
                if nblk == 0:
                    continue
                gt = gpool.tile([P, meta['maxblk'], P], f16,
                                tag=f"g{tg}{b}", name=f"g{tg}{b}_{it}_{g}")
                io = int(idx_off[b, g])
                nc.gpsimd.dma_gather(
                    out_ap=gt[:, :nblk, :],
                    in_ap=tab[b * BUCKET:(b + 1) * BUCKET, :],
                    idxs_ap=idx_sb[:, io:io + 8 * nblk],
                    num_idxs=nblk * P, num_idxs_reg=nblk * P,
                    elem_size=D, single_packet=(nblk * P <= 1024),
                    queue_num=b % 4)
                g_tiles[b] = gt

        msg_ps = msgps.tile([P, W], f32, tag="msg", name=f"m{tg}{it}_{w}")
        nch = int(ncc[:, w].sum())
        ci = 0
        for b in range(nb):
            for j in range(int(ncc[b, w])):
                s_t = spool.tile([P, W], f16, tag="s",
                                 name=f"s{tg}{it}_{w}_{ci}")
                nc.vector.tensor_scalar(
                    out=s_t[:, :T], in0=iota_sb[:, :T],
                    scalar1=tl_sb[:, int(tl_off[b, w]) + j, None],
                    scalar2=None, op0=ALU.is_equal)
                nc.tensor.matmul(
                    out=msg_ps[:, :T],
                    lhsT=g_tiles[b][:, int(blk_off[b, w]) + j, :],
                    rhs=s_t[:, :T],
                    start=(ci == 0), stop=(ci == nch - 1))
                ci += 1

        msg_sb = sb.tile([P, W], f16, tag="msg_sb", name=f"ms{tg}{it}_{w}")
        nc.scalar.copy(out=msg_sb[:, :T], in_=msg_ps[:, :T])

        hp = sb.tile([P, W], f16, tag="hp", name=f"hp{tg}{it}_{w}")
        nc.sync.dma_start(out=hp[:, :T], in_=h_col[:, w * W:w * W + T])

        gates = []
        for gi, fn in enumerate((AF.Sigmoid, AF.Sigmoid, AF.Tanh, AF.Sigmoid)):
            z_ps = zps.tile([P, W], f32, tag="z", name=f"z{tg}{it}_{w}_{gi}")
            nc.tensor.matmul(out=z_ps[:, :T],
                             lhsT=wih_sb[:, gi * P:(gi + 1) * P],
                             rhs=msg_sb[:, :T], start=True, stop=False)
            nc.tensor.matmul(out=z_ps[:, :T],
                             lhsT=whh_sb[:, gi * P:(gi + 1) * P],
                             rhs=hp[:, :T], start=False, stop=True)
            ga = sb.tile([P, W], f16, tag=f"gate{gi}",
                         name=f"ga{tg}{it}_{w}_{gi}")
            nc.scalar.activation(out=ga[:, :T], in_=z_ps[:, :T], func=fn,
                                 bias=bias_sb[:, gi, None])
            gates.append(ga)
        i_g, f_g, g_g, o_g = gates

        csl = c_sb[:, w * W:w * W + T]
        if first:
            nc.vector.tensor_tensor(out=csl, in0=i_g[:, :T], in1=g_g[:, :T],
                                    op=ALU.mult)
        else:
            fc = sb.tile([P, W], f16, tag="fc", name=f"fc{tg}{it}_{w}")
            nc.vector.tensor_tensor(out=fc[:, :T], in0=f_g[:, :T], in1=csl,
                                    op=ALU.mult)
            ig = sb.tile([P, W], f16, tag="ig", name=f"ig{tg}{it}_{w}")
            nc.vector.tensor_tensor(out=ig[:, :T], in0=i_g[:, :T],
                                    in1=g_g[:, :T], op=ALU.mult)
            nc.vector.tensor_tensor(out=csl, in0=fc[:, :T], in1=ig[:, :T],
                                    op=ALU.add)
        tc_t = sb.tile([P, W], f16, tag="tanc", name=f"tc{tg}{it}_{w}")
        nc.scalar.activation(out=tc_t[:, :T], in_=csl, func=AF.Tanh)
        hn = sb.tile([P, W], f16, tag="hn", name=f"hn{tg}{it}_{w}")
        nc.vector.tensor_tensor(out=hn[:, :T], in0=o_g[:, :T],
                                in1=tc_t[:, :T], op=ALU.mult)
        nc.sync.dma_start(out=h_col[:, w * W:w * W + T], in_=hn[:, :T])

        if not last_v:
            tr_ps = trps.tile([P, W], f16, tag="tr", name=f"tr{tg}{it}_{w}")
            nbk = (T + P - 1) // P
            for j in range(nbk):
                nj = min(P, T - j * P)
                nc.tensor.transpose(out=tr_ps[:nj, j * P:j * P + P],
                                    in_=hn[:, j * P:j * P + nj],
                                    identity=ident_sb[:])
            hrow = sb.tile([P, W // P, P], f16, tag="hrow",
                           name=f"hr{tg}{it}_{w}")
            nc.vector.tensor_copy(
                out=hrow[:, :nbk, :],
                in_=tr_ps[:].rearrange("p (b q) -> p b q", q=P)[:, :nbk, :])
            _store_rows(nc, row_shard, w * W, T, hrow[:])

    if not last_v:
        nc.sync.dma_start(out=c_col[:, :], in_=c_sb[:, :NT])


def build_program(me, mv, xvt_cols):
    nc = bacc.Bacc(num_devices=CORES, num_swdge_queues=4)

    emb16 = nc.dram_tensor("emb16", [VOCAB + 1, D], f16, kind="ExternalInput")
    evec_in = nc.dram_tensor("evec_in", [P, W], f16, kind="ExternalInput")
    wihe_in = nc.dram_tensor("wihe", [P, 4 * P], f16, kind="ExternalInput")
    whhe_in = nc.dram_tensor("whhe", [P, 4 * P], f16, kind="ExternalInput")
    wihv_in = nc.dram_tensor("wihv", [P, 4 * P], f16, kind="ExternalInput")
    whhv_in = nc.dram_tensor("whhv", [P, 4 * P], f16, kind="ExternalInput")
    biase_in = nc.dram_tensor("biase", [P, 4], f32, kind="ExternalInput")
    biasv_in = nc.dram_tensor("biasv", [P, 4], f32, kind="ExternalInput")
    wout_in = nc.dram_tensor("woutt", [P, VOCAB], f16, kind="ExternalInput")
    bout_in = nc.dram_tensor("boutr", [P, VOCAB], f32, kind="ExternalInput")
    iota_in = nc.dram_tensor("iota", [P, W], f16, kind="ExternalInput")
    eidx_in = nc.dram_tensor("eidx", [P, me['idx_cols']], i16,
                             kind="ExternalInput")
    etl_in = nc.dram_tensor("etl", [P, me['tl_cols']], f32,
                            kind="ExternalInput")
    vidx_in = nc.dram_tensor("vidx", [P, mv['idx_cols']], i16,
                             kind="ExternalInput")
    vtl_in = nc.dram_tensor("vtl", [P, mv['tl_cols']], f32,
                            kind="ExternalInput")
    xvt_in = nc.dram_tensor("xvt", [P, xvt_cols], i16, kind="ExternalInput")
    xvc_cols = 12544 // 16  # 784: 12500 idxs padded to 98 blocks
    xvc_in = nc.dram_tensor("xvc", [P, xvc_cols], i16, kind="ExternalInput")

    logits = nc.dram_tensor("logits", [VK, VOCAB], f32, kind="ExternalOutput")

    hv_tab = nc.dram_tensor("hv_tab", [V, D], f16, addr_space="Shared")
    he_tab = nc.dram_tensor("he_tab", [E, D], f16, addr_space="Shared")
    hv_row = nc.dram_tensor("hv_row", [VK, D], f16)
    he_row = nc.dram_tensor("he_row", [EK, D], f16)
    hv_col = nc.dram_tensor("hv_col", [P, VK], f16)
    he_col = nc.dram_tensor("he_col", [P, EK], f16)
    cv_col = nc.dram_tensor("cv_col", [P, VK], f16)
    ce_col = nc.dram_tensor("ce_col", [P, EK], f16)

    rg = [list(range(CORES))]

    with TileContext(nc) as tc:
        with tc.tile_pool(name="const", bufs=1) as cpool:
            def load_const(name, src, shape, dt):
                t = cpool.tile(shape, dt, name=name)
                nc.sync.dma_start(out=t[:], in_=src[:, :])
                return t

            wihe_sb = load_const("wihe_sb", wihe_in, [P, 4 * P], f16)
            whhe_sb = load_const("whhe_sb", whhe_in, [P, 4 * P], f16)
            wihv_sb = load_const("wihv_sb", wihv_in, [P, 4 * P], f16)
            whhv_sb = load_const("whhv_sb", whhv_in, [P, 4 * P], f16)
            biase_sb = load_const("biase_sb", biase_in, [P, 4], f32)
            biasv_sb = load_const("biasv_sb", biasv_in, [P, 4], f32)
            iota_sb = load_const("iota_sb", iota_in, [P, W], f16)
            eidx_sb = load_const("eidx_sb", eidx_in, [P, me['idx_cols']], i16)
            etl_sb = load_const("etl_sb", etl_in, [P, me['tl_cols']], f32)
            vidx_sb = load_const("vidx_sb", vidx_in, [P, mv['idx_cols']], i16)
            vtl_sb = load_const("vtl_sb", vtl_in, [P, mv['tl_cols']], f32)
            ident_sb = cpool.tile([P, P], f16, name="ident_sb")
            make_identity(nc, ident_sb[:])

            # ---------------- init ----------------
            with tc.tile_pool(name="initp", bufs=2) as ip:
                xvt_sb = ip.tile([P, xvt_cols], i16, name="xvt_sb", bufs=1)
                nc.sync.dma_start(out=xvt_sb[:], in_=xvt_in[:, :])
                col = 0
                for ci, (start, nrows) in enumerate(INIT_CHUNKS):
                    nblk = (nrows + P - 1) // P
                    gt = ip.tile([P, 100, P], f16, tag="initg",
                                 name=f"ig{start}")
                    nc.gpsimd.dma_gather(
                        out_ap=gt[:, :nblk, :], in_ap=emb16[:, :],
                        idxs_ap=xvt_sb[:, col:col + 8 * nblk],
                        num_idxs=nblk * P, num_idxs_reg=nblk * P,
                        elem_size=D, single_packet=(nblk * P <= 1024),
                        queue_num=ci % 4)
                    _store_rows(nc, hv_tab, start, nrows, gt[:])
                    col += 8 * nblk

                xvc_sb = ip.tile([P, xvc_cols], i16, name="xvc_sb", bufs=1)
                nc.sync.dma_start(out=xvc_sb[:], in_=xvc_in[:, :])
                nci = 16 * xvc_cols  # 12544
                hvc0 = ip.tile([P, 1, nci], f16, name="hvc0", bufs=1)
                nc.gpsimd.dma_gather(
                    out_ap=hvc0[:], in_ap=emb16[:, :], idxs_ap=xvc_sb[:],
                    num_idxs=nci, num_idxs_reg=nci, elem_size=D,
                    transpose=True, single_packet=(nci <= 1024),
                    queue_num=1)
                nc.sync.dma_start(out=hv_col[:, :], in_=hvc0[:, 0, :VK])

                # he_col: broadcast edge_vec by doubling
                nc.sync.dma_start(out=he_col[:, 0:W], in_=evec_in[:, :])
                span = W
                while span < EK:
                    n = min(span, EK - span)
                    nc.sync.dma_start(out=he_col[:, span:span + n],
                                      in_=he_col[:, 0:n])
                    span += n

            # ---------------- main loop ----------------
            with (
                tc.tile_pool(name="sb", bufs=2) as sb,
                tc.tile_pool(name="spool", bufs=4) as spool,
                tc.tile_pool(name="ge", bufs=2) as ge,
                tc.tile_pool(name="gv", bufs=2) as gv,
                tc.tile_pool(name="cres", bufs=1) as cres,
                tc.tile_pool(name="msgps", bufs=2, space="PSUM") as msgps,
                tc.tile_pool(name="zps", bufs=4, space="PSUM") as zps,
                tc.tile_pool(name="trps", bufs=2, space="PSUM") as trps,
            ):
                pools = dict(sb=sb, s=spool, msg=msgps, z=zps, tr=trps,
                             c=cres, ge=ge, gv=gv)
                cfg_e = dict(tab=hv_tab, h_col=he_col, c_col=ce_col,
                             row_shard=he_row, idx_sb=eidx_sb, tl_sb=etl_sb,
                             wih_sb=wihe_sb, whh_sb=whhe_sb,
                             bias_sb=biase_sb, iota_sb=iota_sb,
                             ident_sb=ident_sb, tag='e')
                cfg_v = dict(tab=he_tab, h_col=hv_col, c_col=cv_col,
                             row_shard=hv_row, idx_sb=vidx_sb, tl_sb=vtl_sb,
                             wih_sb=wihv_sb, whh_sb=whhv_sb,
                             bias_sb=biasv_sb, iota_sb=iota_sb,
                             ident_sb=ident_sb, tag='v')
                stage = DEBUG_STAGE
                for it in range(ITERS):
                    if stage == 'init':
                        break
                    if it > 0:
                        for j in range(NB_E):
                            nc.gpsimd.collective_compute(
                                "AllGather", ALU.bypass, replica_groups=rg,
                                ins=[hv_row[j * SUB_V:(j + 1) * SUB_V, :].opt()],
                                outs=[hv_tab[j * BUCKET:(j + 1) * BUCKET, :].opt()])
                    _build_phase(nc, pools, cfg_e, me, it,
                                 first=(it == 0), last_v=False)
                    if stage == 'edge':
                        break
                    for j in range(NB_V):
                        nc.gpsimd.collective_compute(
                            "AllGather", ALU.bypass, replica_groups=rg,
                            ins=[he_row[j * SUB_E:(j + 1) * SUB_E, :].opt()],
                            outs=[he_tab[j * BUCKET:(j + 1) * BUCKET, :].opt()])
                    if stage == 'agv':
                        break
                    _build_phase(nc, pools, cfg_v, mv, it,
                                 first=(it == 0), last_v=(it == ITERS - 1))

            # ---------------- logits ----------------
            with (
                tc.tile_pool(name="lsb", bufs=3) as lsb,
                tc.tile_pool(name="lcp", bufs=1) as lcp,
                tc.tile_pool(name="lps", bufs=2, space="PSUM") as lps,
            ):
                wout_sb = lcp.tile([P, VOCAB], f16, name="wout_sb")
                nc.sync.dma_start(out=wout_sb[:], in_=wout_in[:, :])
                bout_sb = lcp.tile([P, VOCAB], f32, name="bout_sb")
                nc.sync.dma_start(out=bout_sb[:], in_=bout_in[:, :])
                hvc_sb = lcp.tile([P, VK], f16, name="hvc_sb")
                nc.sync.dma_start(out=hvc_sb[:], in_=hv_col[:, :])
                nch = (VK + P - 1) // P
                for ch in range(nch):
                    n = min(P, VK - ch * P)
                    lp = lps.tile([P, VOCAB], f32, tag="lp", name=f"lp{ch}")
                    nc.tensor.matmul(out=lp[:n, :W],
                                     lhsT=hvc_sb[:, ch * P:ch * P + n],
                                     rhs=wout_sb[:, :W],
                                     start=True, stop=True)
                    nc.tensor.matmul(out=lp[:n, W:VOCAB],
                                     lhsT=hvc_sb[:, ch * P:ch * P + n],
                                     rhs=wout_sb[:, W:VOCAB],
                                     start=True, stop=True)
                    ob = lsb.tile([P, VOCAB], f32, tag="ob", name=f"ob{ch}")
                    nc.vector.tensor_tensor(out=ob[:n, :], in0=lp[:n, :],
                                            in1=bout_sb[:n, :], op=ALU.add)
                    nc.sync.dma_start(out=logits[ch * P:ch * P + n, :],
                                      in_=ob[:n, :])

    nc.compile()
    return nc


# --------------------------------------------------------------------------
# entry point
# --------------------------------------------------------------------------

def _prepare(inputs):
    rows = np.asarray(inputs['adj_rows']).astype(np.int64)
    cols = np.asarray(inputs['adj_cols']).astype(np.int64)
    x_v = np.asarray(inputs['x_v']).astype(np.int64)

    me = _prep_phase(rows, cols, EK, VK, NB_E, NW_E, WG_E)
    mv = _prep_phase(cols, rows, VK, EK, NB_V, NW_V, WG_V)
    xvt_arr, xvt_cols = _prep_init(x_v)

    emb = np.asarray(inputs['emb'], np.float32)
    emb16 = np.ascontiguousarray(emb.astype(np.float16))
    evec = (np.asarray(inputs['edge_init_w'], np.float32)[:, 0]
            + np.asarray(inputs['edge_init_b'], np.float32))
    evec_tile = np.ascontiguousarray(
        np.tile(evec.astype(np.float16)[:, None], (1, W)))

    def wt(name):
        return np.ascontiguousarray(
            np.asarray(inputs[name], np.float32).T.astype(np.float16))

    def bias(ih, hh):
        b = (np.asarray(inputs[ih], np.float32)
             + np.asarray(inputs[hh], np.float32))
        return np.ascontiguousarray(b.reshape(4, P).T)

    wout_t = np.ascontiguousarray(
        np.asarray(inputs['Wout'], np.float32).T.astype(np.float16))
    bout_rep = np.ascontiguousarray(
        np.tile(np.asarray(inputs['bout'], np.float32)[None, :], (P, 1)))
    iota = np.ascontiguousarray(
        np.tile(np.arange(W, dtype=np.float32).astype(np.float16), (P, 1)))

    common = dict(
        emb16=emb16, evec_in=evec_tile,
        wihe=wt('Wih_e'), whhe=wt('Whh_e'),
        wihv=wt('Wih_v'), whhv=wt('Whh_v'),
        biase=bias('bih_e', 'bhh_e'), biasv=bias('bih_v', 'bhh_v'),
        woutt=wout_t, boutr=bout_rep, iota=iota, xvt=xvt_arr,
    )
    in_maps = []
    for k in range(CORES):
        xs = x_v[k * VK:(k + 1) * VK].astype(np.int16)
        xvc = np.concatenate([xs, np.full(12544 - VK, xs[-1], np.int16)])
        m = dict(common)
        m.update(
            eidx=me['idx_arrs'][k], etl=me['tl_arrs'][k],
            vidx=mv['idx_arrs'][k], vtl=mv['tl_arrs'][k],
            xvc=_wrap16(xvc, 12544 // 16),
        )
        in_maps.append(m)
    return me, mv, xvt_cols, in_maps


def run_spmd(inputs, **kw):
    me, mv, xvt_cols, in_maps = _prepare(inputs)
    key = (me['ncc'].tobytes(), mv['ncc'].tobytes(), xvt_cols,
           me['gblocks'].tobytes(), mv['gblocks'].tobytes(),
           ITERS, DEBUG_STAGE)
    if key not in _CACHE:
        _CACHE[key] = build_program(me, mv, xvt_cols)
    nc = _CACHE[key]
    return bass_utils.run_bass_kernel_spmd(
        nc, in_maps, core_ids=list(range(CORES)), **kw)


def kernel(**inputs) -> np.ndarray:
    res = run_spmd(inputs)
    out = np.concatenate([res.results[k]['logits'] for k in range(CORES)], 0)
    return out.astype(np.float32)



# revision 15
# speedup vs baseline: 1.4091x; 1.1309x over previous
"""Bass/Trainium2 kernel for nn_BipartiteGNN: 6 rounds of bipartite LSTM
message passing + output projection, distributed over 8 NeuronCores.

Distribution:
 - Edges/vertices sharded 8 ways (contiguous blocks). The gathered h
   tables are replicated per core in a subrange-permuted row layout
   (row v -> j*25000 + rank*3125 + i%3125, j=i//3125) so each of the
   chunked AllGathers rebuilds one contiguous 25000-row gather bucket
   (int16 gather indices) and overlaps with compute.
 - segment_sum: dma_gather of source rows (entries on partitions, 256B
   fp16 rows) + one-hot selection matmul per 128-entry chunk,
   accumulating into a column-major [128(D), 512] PSUM msg window. The
   selection matrix is built on DVE via tensor_scalar is_equal against
   an iota row (4x fp16 mode).
 - LSTM runs column-major (D on partitions) in fp16 with f32 PSUM; c
   state is fp16, SBUF-resident per phase. h is written back both
   column-major (local recurrent input) and, via PE transpose,
   row-major into the AllGather input shard.
"""
import sys
import numpy as np

sys.path.insert(0, '/opt/trn_rl_repo')

import concourse.bacc as bacc  # noqa: E402
import concourse.mybir as mybir  # noqa: E402
from concourse.tile import TileContext  # noqa: E402
from concourse.masks import make_identity  # noqa: E402
from concourse import bass_utils  # noqa: E402

P = 128
D = 128
V = 100000
E = 200000
VOCAB = 1000
ITERS = 6
CORES = 8
W = 512

EK = E // CORES        # 25000
VK = V // CORES        # 12500
BUCKET = 25000         # rows per gather bucket in both tables

NB_E = 4               # buckets over the (permuted) vertex table
NB_V = 8               # buckets over the (permuted) edge table
SUB_V = VK // NB_E     # 3125
SUB_E = EK // NB_V     # 3125
WG_E = 6
WG_V = 4

NW_E = (EK + W - 1) // W   # 49
NW_V = (VK + W - 1) // W   # 25

f16 = mybir.dt.float16
f32 = mybir.dt.float32
i16 = mybir.dt.int16

AF = mybir.ActivationFunctionType
ALU = mybir.AluOpType

_CACHE = {}
DEBUG_STAGE = 'full'


# --------------------------------------------------------------------------
# host-side index preprocessing
# --------------------------------------------------------------------------

def _perm_pos(glob, shard, nb):
    sub = shard // nb
    rank = glob // shard
    i = glob - rank * shard
    j = i // sub
    return j * (CORES * sub) + rank * sub + (i - j * sub)


def _wrap16(vals, ncols):
    buf = np.zeros(ncols * 16, np.int16)
    buf[:len(vals)] = vals
    return np.tile(buf.reshape(ncols, 16).T, (8, 1))


def _prep_phase(tgt, src, n_t_local, src_shard, nb, nw, wg):
    src_perm = _perm_pos(src, src_shard, nb)
    bucket = src_perm // BUCKET
    sloc = (src_perm - bucket * BUCKET).astype(np.int64)

    percore = []
    for k in range(CORES):
        m = (tgt // n_t_local) == k
        t = tgt[m] - k * n_t_local
        b = bucket[m]
        s = sloc[m]
        w = t // W
        order = np.lexsort((t, w, b))
        percore.append((b[order], w[order], t[order], s[order]))

    sizes = np.zeros((CORES, nb, nw), np.int64)
    for k in range(CORES):
        b, w, _, _ = percore[k]
        np.add.at(sizes[k], (b, w), 1)
    ncc = (sizes.max(axis=0) + P - 1) // P
    for w in range(nw):
        if ncc[:, w].sum() == 0:
            ncc[0, w] = 1

    ng = (nw + wg - 1) // wg
    gblocks = np.zeros((nb, ng), np.int64)
    for b in range(nb):
        for g in range(ng):
            gblocks[b, g] = ncc[b, g * wg:(g + 1) * wg].sum()
    maxblk = int(gblocks.max())

    idx_off = np.zeros((nb, ng), np.int64)
    o = 0
    for b in range(nb):
        for g in range(ng):
            idx_off[b, g] = o
            o += 8 * int(gblocks[b, g])
    idx_cols = int(o)

    tl_off = np.zeros((nb, nw), np.int64)
    o = 0
    for w in range(nw):
        for b in range(nb):
            tl_off[b, w] = o
            o += int(ncc[b, w])
    tl_cols = int(o)

    blk_off = np.zeros((nb, nw), np.int64)
    for b in range(nb):
        for g in range(ng):
            o = 0
            for w in range(g * wg, min((g + 1) * wg, nw)):
                blk_off[b, w] = o
                o += int(ncc[b, w])

    idx_arrs, tl_arrs = [], []
    for k in range(CORES):
        b_a, w_a, t_a, s_a = percore[k]
        idx_full = np.zeros(idx_cols * 16, np.int16)
        tl_full = np.full(tl_cols * P, -1.0, np.float32)
        cell_start = {}
        pos = 0
        for b in range(nb):
            for w in range(nw):
                cell_start[(b, w)] = pos
                pos += int(sizes[k, b, w])
        for b in range(nb):
            for g in range(ng):
                base = int(idx_off[b, g]) * 16
                cur = 0
                for w in range(g * wg, min((g + 1) * wg, nw)):
                    cs = cell_start[(b, w)]
                    cnt = int(sizes[k, b, w])
                    L = int(ncc[b, w]) * P
                    if L == 0:
                        continue
                    sl = s_a[cs:cs + cnt]
                    padded = np.full(L, sl[0] if cnt else 0, np.int16)
                    padded[:cnt] = sl
                    idx_full[base + cur: base + cur + L] = padded
                    tlv = np.full(L, -1.0, np.float32)
                    tlv[:cnt] = (t_a[cs:cs + cnt] - w * W).astype(np.float32)
                    tb = int(tl_off[b, w]) * P
                    tl_full[tb:tb + L] = tlv
                    cur += L
        idx_arrs.append(np.tile(idx_full.reshape(idx_cols, 16).T, (8, 1)))
        tl_arrs.append(np.ascontiguousarray(tl_full.reshape(tl_cols, P).T))

    return dict(
        ncc=ncc, ng=ng, wg=wg, nb=nb, nw=nw, maxblk=maxblk,
        gblocks=gblocks, idx_off=idx_off, idx_cols=idx_cols,
        tl_off=tl_off, tl_cols=tl_cols, blk_off=blk_off,
        idx_arrs=idx_arrs, tl_arrs=tl_arrs, n_t_local=n_t_local,
    )


# init chunks for building hv_tab (perm layout): (start_row, nrows)
INIT_CHUNKS = [(i * 12800, 12800) for i in range(7)] + [(89600, 10400)]


def _prep_init(x_v):
    """idx arrays (into emb) for hv_tab init gathers, in perm order."""
    perm = _perm_pos(np.arange(V, dtype=np.int64), VK, NB_E)
    inv = np.empty(V, np.int64)
    inv[perm] = np.arange(V, dtype=np.int64)
    xv_of_pos = x_v[inv]                      # emb row for table position p
    pieces = []
    for start, nrows in INIT_CHUNKS:
        nblk = (nrows + P - 1) // P
        vals = np.full(nblk * P, 0, np.int16)
        vals[:nrows] = xv_of_pos[start:start + nrows]
        if nrows < nblk * P:
            vals[nrows:] = vals[nrows - 1]
        pieces.append(vals)
    allv = np.concatenate(pieces)
    ncols = len(allv) // 16
    return _wrap16(allv, ncols), ncols


# --------------------------------------------------------------------------
# device program
# --------------------------------------------------------------------------

def _store_rows(nc, dram, row0, nrows, sb3):
    """[128, nblk, 128] fp16 tile -> dram rows [row0, row0+nrows)."""
    nbf = nrows // P
    rem = nrows - nbf * P
    if nbf:
        nc.sync.dma_start(
            out=dram[row0:row0 + nbf * P, :].rearrange("(b p) d -> p b d", p=P),
            in_=sb3[:, :nbf, :])
    if rem:
        nc.sync.dma_start(
            out=dram[row0 + nbf * P: row0 + nrows, :],
            in_=sb3[:rem, nbf, :])


def _build_phase(nc, pools, cfg, meta, it, first, last_v):
    sb, spool, msgps, zps, trps, cpool_c = (
        pools['sb'], pools['s'], pools['msg'], pools['z'], pools['tr'],
        pools['c'])
    skip_h = first and cfg['tag'] == 'e'   # h0 folded into bias0
    gpool = pools['g' + cfg['tag']]
    NT = meta['n_t_local']
    nw, nb, wg = meta['nw'], meta['nb'], meta['wg']
    ncc, blk_off, tl_off = meta['ncc'], meta['blk_off'], meta['tl_off']
    gblocks, idx_off = meta['gblocks'], meta['idx_off']
    tab, h_col, c_col = cfg['tab'], cfg['h_col'], cfg['c_col']
    row_shard = cfg['row_shard']
    idx_sb, tl_sb = cfg['idx_sb'], cfg['tl_sb']
    wih_sb, whh_sb = cfg['wih_sb'], cfg['whh_sb']
    bias_sb = cfg['bias0_sb'] if skip_h else cfg['bias_sb']
    iota_sb, ident_sb = cfg['iota_sb'], cfg['ident_sb']
    tg = cfg['tag']

    c_sb = cpool_c.tile([P, EK], f16, tag="c_res", name=f"c{tg}{it}")
    if not first:
        nc.sync.dma_start(out=c_sb[:, :NT], in_=c_col[:, :])

    g_tiles = {}
    for w in range(nw):
        T = min(W, NT - w * W)
        g = w // wg
        if w % wg == 0:
            for b in range(nb):
                nblk = int(gblocks[b, g])
                if nblk == 0:
                    continue
                gt = gpool.tile([P, meta['maxblk'], P], f16,
                                tag=f"g{tg}{b}", name=f"g{tg}{b}_{it}_{g}")
                io = int(idx_off[b, g])
                nc.gpsimd.dma_gather(
                    out_ap=gt[:, :nblk, :],
                    in_ap=tab[b * BUCKET:(b + 1) * BUCKET, :],
                    idxs_ap=idx_sb[:, io:io + 8 * nblk],
                    num_idxs=nblk * P, num_idxs_reg=nblk * P,
                    elem_size=D, single_packet=(nblk * P <= 1024),
                    queue_num=b % 4)
                g_tiles[b] = gt

        msg_ps = msgps.tile([P, W], f32, tag="msg", name=f"m{tg}{it}_{w}")
        nch = int(ncc[:, w].sum())
        ci = 0
        for b in range(nb):
            for j in range(int(ncc[b, w])):
                s_t = spool.tile([P, W], f16, tag="s",
                                 name=f"s{tg}{it}_{w}_{ci}")
                nc.vector.tensor_scalar(
                    out=s_t[:, :T], in0=iota_sb[:, :T],
                    scalar1=tl_sb[:, int(tl_off[b, w]) + j, None],
                    scalar2=None, op0=ALU.is_equal)
                nc.tensor.matmul(
                    out=msg_ps[:, :T],
                    lhsT=g_tiles[b][:, int(blk_off[b, w]) + j, :],
                    rhs=s_t[:, :T],
                    start=(ci == 0), stop=(ci == nch - 1))
                ci += 1

        msg_sb = sb.tile([P, W], f16, tag="msg_sb", name=f"ms{tg}{it}_{w}")
        nc.scalar.copy(out=msg_sb[:, :T], in_=msg_ps[:, :T])

        if not skip_h:
            hp = sb.tile([P, W], f16, tag="hp", name=f"hp{tg}{it}_{w}")
            nc.sync.dma_start(out=hp[:, :T], in_=h_col[:, w * W:w * W + T])

        gates = []
        for gi, fn in enumerate((AF.Sigmoid, AF.Sigmoid, AF.Tanh, AF.Sigmoid)):
            z_ps = zps.tile([P, W], f32, tag="z", name=f"z{tg}{it}_{w}_{gi}")
            nc.tensor.matmul(out=z_ps[:, :T],
                             lhsT=wih_sb[:, gi * P:(gi + 1) * P],
                             rhs=msg_sb[:, :T], start=True, stop=skip_h)
            if not skip_h:
                nc.tensor.matmul(out=z_ps[:, :T],
                                 lhsT=whh_sb[:, gi * P:(gi + 1) * P],
                                 rhs=hp[:, :T], start=False, stop=True)
            ga = sb.tile([P, W], f16, tag=f"gate{gi}",
                         name=f"ga{tg}{it}_{w}_{gi}")
            nc.scalar.activation(out=ga[:, :T], in_=z_ps[:, :T], func=fn,
                                 bias=bias_sb[:, gi, None])
            gates.append(ga)
        i_g, f_g, g_g, o_g = gates

        csl = c_sb[:, w * W:w * W + T]
        if first:
            nc.vector.tensor_tensor(out=csl, in0=i_g[:, :T], in1=g_g[:, :T],
                                    op=ALU.mult)
        else:
            fc = sb.tile([P, W], f16, tag="fc", name=f"fc{tg}{it}_{w}")
            nc.vector.tensor_tensor(out=fc[:, :T], in0=f_g[:, :T], in1=csl,
                                    op=ALU.mult)
            ig = sb.tile([P, W], f16, tag="ig", name=f"ig{tg}{it}_{w}")
            nc.vector.tensor_tensor(out=ig[:, :T], in0=i_g[:, :T],
                                    in1=g_g[:, :T], op=ALU.mult)
            nc.vector.tensor_tensor(out=csl, in0=fc[:, :T], in1=ig[:, :T],
                                    op=ALU.add)
        tc_t = sb.tile([P, W], f16, tag="tanc", name=f"tc{tg}{it}_{w}")
        nc.scalar.activation(out=tc_t[:, :T], in_=csl, func=AF.Tanh)
        hn = sb.tile([P, W], f16, tag="hn", name=f"hn{tg}{it}_{w}")
        nc.vector.tensor_tensor(out=hn[:, :T], in0=o_g[:, :T],
                                in1=tc_t[:, :T], op=ALU.mult)
        nc.sync.dma_start(out=h_col[:, w * W:w * W + T], in_=hn[:, :T])

        if not last_v:
            tr_ps = trps.tile([P, W], f16, tag="tr", name=f"tr{tg}{it}_{w}")
            nbk = (T + P - 1) // P
            for j in range(nbk):
                nj = min(P, T - j * P)
                nc.tensor.transpose(out=tr_ps[:nj, j * P:j * P + P],
                                    in_=hn[:, j * P:j * P + nj],
                                    identity=ident_sb[:])
            hrow = sb.tile([P, W // P, P], f16, tag="hrow",
                           name=f"hr{tg}{it}_{w}")
            nc.vector.tensor_copy(
                out=hrow[:, :nbk, :],
                in_=tr_ps[:].rearrange("p (b q) -> p b q", q=P)[:, :nbk, :])
            _store_rows(nc, row_shard, w * W, T, hrow[:])

    if not last_v:
        nc.sync.dma_start(out=c_col[:, :], in_=c_sb[:, :NT])


def build_program(me, mv):
    nc = bacc.Bacc(num_devices=CORES, num_swdge_queues=4)

    embp_in = nc.dram_tensor("embp", [1024, D], f16, kind="ExternalInput")
    wihe_in = nc.dram_tensor("wihe", [P, 4 * P], f16, kind="ExternalInput")
    whhe_in = nc.dram_tensor("whhe", [P, 4 * P], f16, kind="ExternalInput")
    wihv_in = nc.dram_tensor("wihv", [P, 4 * P], f16, kind="ExternalInput")
    whhv_in = nc.dram_tensor("whhv", [P, 4 * P], f16, kind="ExternalInput")
    biase_in = nc.dram_tensor("biase", [P, 4], f32, kind="ExternalInput")
    biase0_in = nc.dram_tensor("biase0", [P, 4], f32, kind="ExternalInput")
    biasv_in = nc.dram_tensor("biasv", [P, 4], f32, kind="ExternalInput")
    wout_in = nc.dram_tensor("woutt", [P, VOCAB], f16, kind="ExternalInput")
    bout_in = nc.dram_tensor("boutr", [P, VOCAB], f32, kind="ExternalInput")
    iota_in = nc.dram_tensor("iota", [P, W], f16, kind="ExternalInput")
    eidx_in = nc.dram_tensor("eidx", [P, me['idx_cols']], i16,
                             kind="ExternalInput")
    etl_in = nc.dram_tensor("etl", [P, me['tl_cols']], f32,
                            kind="ExternalInput")
    vidx_in = nc.dram_tensor("vidx", [P, mv['idx_cols']], i16,
                             kind="ExternalInput")
    vtl_in = nc.dram_tensor("vtl", [P, mv['tl_cols']], f32,
                            kind="ExternalInput")
    xvb_in = nc.dram_tensor("xvb", [P, VK], f16, kind="ExternalInput")
    pcol_in = nc.dram_tensor("pcol", [P, 8], f32, kind="ExternalInput")

    logits = nc.dram_tensor("logits", [VK, VOCAB], f32, kind="ExternalOutput")

    hv_tab = nc.dram_tensor("hv_tab", [V, D], f16, addr_space="Shared")
    he_tab = nc.dram_tensor("he_tab", [E, D], f16, addr_space="Shared")
    hv_row = nc.dram_tensor("hv_row", [VK, D], f16)
    he_row = nc.dram_tensor("he_row", [EK, D], f16)
    hv_col = nc.dram_tensor("hv_col", [P, VK], f16)
    he_col = nc.dram_tensor("he_col", [P, EK], f16)
    cv_col = nc.dram_tensor("cv_col", [P, VK], f16)
    ce_col = nc.dram_tensor("ce_col", [P, EK], f16)

    rg = [list(range(CORES))]

    with TileContext(nc) as tc:
        with tc.tile_pool(name="const", bufs=1) as cpool:
            def load_const(name, src, shape, dt):
                t = cpool.tile(shape, dt, name=name)
                nc.sync.dma_start(out=t[:], in_=src[:, :])
                return t

            wihe_sb = load_const("wihe_sb", wihe_in, [P, 4 * P], f16)
            whhe_sb = load_const("whhe_sb", whhe_in, [P, 4 * P], f16)
            wihv_sb = load_const("wihv_sb", wihv_in, [P, 4 * P], f16)
            whhv_sb = load_const("whhv_sb", whhv_in, [P, 4 * P], f16)
            biase_sb = load_const("biase_sb", biase_in, [P, 4], f32)
            biase0_sb = load_const("biase0_sb", biase0_in, [P, 4], f32)
            biasv_sb = load_const("biasv_sb", biasv_in, [P, 4], f32)
            iota_sb = load_const("iota_sb", iota_in, [P, W], f16)
            eidx_sb = load_const("eidx_sb", eidx_in, [P, me['idx_cols']], i16)
            etl_sb = load_const("etl_sb", etl_in, [P, me['tl_cols']], f32)
            vidx_sb = load_const("vidx_sb", vidx_in, [P, mv['idx_cols']], i16)
            vtl_sb = load_const("vtl_sb", vtl_in, [P, mv['tl_cols']], f32)
            ident_sb = cpool.tile([P, P], f16, name="ident_sb")
            make_identity(nc, ident_sb[:])

            # ---------------- init: h_v = onehot(x_v) @ emb ----------------
            with (
                tc.tile_pool(name="initp", bufs=2) as ip,
                tc.tile_pool(name="initps", bufs=2, space="PSUM") as ips,
            ):
                embt = ip.tile([P, 8, P], f16, name="embt", bufs=1)
                nc.sync.dma_start(
                    out=embt[:],
                    in_=embp_in[:, :].rearrange("(c p) d -> p c d", p=P))
                xvb_sb = ip.tile([P, VK], f16, name="xvb_sb", bufs=1)
                nc.sync.dma_start(out=xvb_sb[:], in_=xvb_in[:, :])
                pcol_sb = ip.tile([P, 8], f32, name="pcol_sb", bufs=1)
                nc.sync.dma_start(out=pcol_sb[:], in_=pcol_in[:, :])
                for w in range(NW_V):
                    T = min(W, VK - w * W)
                    hp_ps = ips.tile([P, W], f32, tag="ips", name=f"ips{w}")
                    for c in range(8):
                        oh = ip.tile([P, W], f16, tag="ioh",
                                     name=f"oh{w}_{c}")
                        nc.vector.tensor_scalar(
                            out=oh[:, :T], in0=xvb_sb[:, w * W:w * W + T],
                            scalar1=pcol_sb[:, c, None], scalar2=None,
                            op0=ALU.is_equal)
                        nc.tensor.matmul(out=hp_ps[:, :T],
                                         lhsT=embt[:, c, :], rhs=oh[:, :T],
                                         start=(c == 0), stop=(c == 7))
                    hn = ip.tile([P, W], f16, tag="ihn", name=f"ihn{w}")
                    nc.scalar.copy(out=hn[:, :T], in_=hp_ps[:, :T])
                    nc.sync.dma_start(out=hv_col[:, w * W:w * W + T],
                                      in_=hn[:, :T])
                    tr_ps = ips.tile([P, W], f16, tag="itr", name=f"itr{w}")
                    nbk = (T + P - 1) // P
                    for j in range(nbk):
                        nj = min(P, T - j * P)
                        nc.tensor.transpose(out=tr_ps[:nj, j * P:j * P + P],
                                            in_=hn[:, j * P:j * P + nj],
                                            identity=ident_sb[:])
                    hrow = ip.tile([P, W // P, P], f16, tag="ihr",
                                   name=f"ihr{w}")
                    nc.vector.tensor_copy(
                        out=hrow[:, :nbk, :],
                        in_=tr_ps[:].rearrange("p (b q) -> p b q",
                                               q=P)[:, :nbk, :])
                    _store_rows(nc, hv_row, w * W, T, hrow[:])

            # ---------------- main loop ----------------
            with (
                tc.tile_pool(name="sb", bufs=2) as sb,
                tc.tile_pool(name="spool", bufs=6) as spool,
                tc.tile_pool(name="ge", bufs=2) as ge,
                tc.tile_pool(name="gv", bufs=2) as gv,
                tc.tile_pool(name="cres", bufs=1) as cres,
                tc.tile_pool(name="msgps", bufs=2, space="PSUM") as msgps,
                tc.tile_pool(name="zps", bufs=4, space="PSUM") as zps,
                tc.tile_pool(name="trps", bufs=2, space="PSUM") as trps,
            ):
                pools = dict(sb=sb, s=spool, msg=msgps, z=zps, tr=trps,
                             c=cres, ge=ge, gv=gv)
                cfg_e = dict(tab=hv_tab, h_col=he_col, c_col=ce_col,
                             row_shard=he_row, idx_sb=eidx_sb, tl_sb=etl_sb,
                             wih_sb=wihe_sb, whh_sb=whhe_sb,
                             bias_sb=biase_sb, bias0_sb=biase0_sb,
                             iota_sb=iota_sb, ident_sb=ident_sb, tag='e')
                cfg_v = dict(tab=he_tab, h_col=hv_col, c_col=cv_col,
                             row_shard=hv_row, idx_sb=vidx_sb, tl_sb=vtl_sb,
                             wih_sb=wihv_sb, whh_sb=whhv_sb,
                             bias_sb=biasv_sb, bias0_sb=biasv_sb,
                             iota_sb=iota_sb, ident_sb=ident_sb, tag='v')
                stage = DEBUG_STAGE
                for it in range(ITERS):
                    if stage == 'init':
                        break
                    for j in range(NB_E):
                        nc.gpsimd.collective_compute(
                            "AllGather", ALU.bypass, replica_groups=rg,
                            ins=[hv_row[j * SUB_V:(j + 1) * SUB_V, :].opt()],
                            outs=[hv_tab[j * BUCKET:(j + 1) * BUCKET, :].opt()])
                    _build_phase(nc, pools, cfg_e, me, it,
                                 first=(it == 0), last_v=False)
                    if stage == 'edge':
                        break
                    for j in range(NB_V):
                        nc.gpsimd.collective_compute(
                            "AllGather", ALU.bypass, replica_groups=rg,
                            ins=[he_row[j * SUB_E:(j + 1) * SUB_E, :].opt()],
                            outs=[he_tab[j * BUCKET:(j + 1) * BUCKET, :].opt()])
                    if stage == 'agv':
                        break
                    _build_phase(nc, pools, cfg_v, mv, it,
                                 first=(it == 0), last_v=(it == ITERS - 1))

            # ---------------- logits ----------------
            with (
                tc.tile_pool(name="lsb", bufs=3) as lsb,
                tc.tile_pool(name="lcp", bufs=1) as lcp,
                tc.tile_pool(name="lps", bufs=2, space="PSUM") as lps,
            ):
                wout_sb = lcp.tile([P, VOCAB], f16, name="wout_sb")
                nc.sync.dma_start(out=wout_sb[:], in_=wout_in[:, :])
                bout_sb = lcp.tile([P, VOCAB], f32, name="bout_sb")
                nc.sync.dma_start(out=bout_sb[:], in_=bout_in[:, :])
                hvc_sb = lcp.tile([P, VK], f16, name="hvc_sb")
                nc.sync.dma_start(out=hvc_sb[:], in_=hv_col[:, :])
                nch = (VK + P - 1) // P
                for ch in range(nch):
                    n = min(P, VK - ch * P)
                    lp = lps.tile([P, VOCAB], f32, tag="lp", name=f"lp{ch}")
                    nc.tensor.matmul(out=lp[:n, :W],
                                     lhsT=hvc_sb[:, ch * P:ch * P + n],
                                     rhs=wout_sb[:, :W],
                                     start=True, stop=True)
                    nc.tensor.matmul(out=lp[:n, W:VOCAB],
                                     lhsT=hvc_sb[:, ch * P:ch * P + n],
                                     rhs=wout_sb[:, W:VOCAB],
                                     start=True, stop=True)
                    ob = lsb.tile([P, VOCAB], f32, tag="ob", name=f"ob{ch}")
                    nc.vector.tensor_tensor(out=ob[:n, :], in0=lp[:n, :],
                                            in1=bout_sb[:n, :], op=ALU.add)
                    nc.sync.dma_start(out=logits[ch * P:ch * P + n, :],
                                      in_=ob[:n, :])

    nc.compile()
    return nc


# --------------------------------------------------------------------------
# entry point
# --------------------------------------------------------------------------

def _prepare(inputs):
    rows = np.asarray(inputs['adj_rows']).astype(np.int64)
    cols = np.asarray(inputs['adj_cols']).astype(np.int64)
    x_v = np.asarray(inputs['x_v']).astype(np.int64)

    me = _prep_phase(rows, cols, EK, VK, NB_E, NW_E, WG_E)
    mv = _prep_phase(cols, rows, VK, EK, NB_V, NW_V, WG_V)

    emb = np.asarray(inputs['emb'], np.float32)
    embp = np.zeros((1024, D), np.float16)
    embp[:VOCAB + 1] = emb.astype(np.float16)
    evec = (np.asarray(inputs['edge_init_w'], np.float32)[:, 0]
            + np.asarray(inputs['edge_init_b'], np.float32))

    def wt(name):
        return np.ascontiguousarray(
            np.asarray(inputs[name], np.float32).T.astype(np.float16))

    def bias(ih, hh, extra=None):
        b = (np.asarray(inputs[ih], np.float32)
             + np.asarray(inputs[hh], np.float32))
        if extra is not None:
            b = b + extra
        return np.ascontiguousarray(b.reshape(4, P).T)

    whhe_evec = np.asarray(inputs['Whh_e'], np.float32) @ evec  # [4D]

    wout_t = np.ascontiguousarray(
        np.asarray(inputs['Wout'], np.float32).T.astype(np.float16))
    bout_rep = np.ascontiguousarray(
        np.tile(np.asarray(inputs['bout'], np.float32)[None, :], (P, 1)))
    iota = np.ascontiguousarray(
        np.tile(np.arange(W, dtype=np.float32).astype(np.float16), (P, 1)))
    pcol = np.ascontiguousarray(
        (np.arange(P)[:, None] + P * np.arange(8)[None, :])
        .astype(np.float32))

    common = dict(
        embp=embp,
        wihe=wt('Wih_e'), whhe=wt('Whh_e'),
        wihv=wt('Wih_v'), whhv=wt('Whh_v'),
        biase=bias('bih_e', 'bhh_e'),
        biase0=bias('bih_e', 'bhh_e', whhe_evec),
        biasv=bias('bih_v', 'bhh_v'),
        woutt=wout_t, boutr=bout_rep, iota=iota, pcol=pcol,
    )
    in_maps = []
    for k in range(CORES):
        xvb = np.ascontiguousarray(np.tile(
            x_v[k * VK:(k + 1) * VK].astype(np.float16)[None, :], (P, 1)))
        m = dict(common)
        m.update(
            eidx=me['idx_arrs'][k], etl=me['tl_arrs'][k],
            vidx=mv['idx_arrs'][k], vtl=mv['tl_arrs'][k],
            xvb=xvb,
        )
        in_maps.append(m)
    return me, mv, in_maps


def run_spmd(inputs, **kw):
    me, mv, in_maps = _prepare(inputs)
    key = (me['ncc'].tobytes(), mv['ncc'].tobytes(),
           me['gblocks'].tobytes(), mv['gblocks'].tobytes(),
           ITERS, DEBUG_STAGE)
    if key not in _CACHE:
        _CACHE[key] = build_program(me, mv)
    nc = _CACHE[key]
    return bass_utils.run_bass_kernel_spmd(
        nc, in_maps, core_ids=list(range(CORES)), **kw)


def kernel(**inputs) -> np.ndarray:
    res = run_spmd(inputs)
    out = np.concatenate([res.results[k]['logits'] for k in range(CORES)], 0)
    return out.astype(np.float32)

